# revision 22
# baseline (speedup 1.0000x reference)
"""Trainium2 Bass kernel for batched attention (B=16, N=2048, D=256).

reference:
  scores = einsum('bnd,bmd->bnm', q, k) / sqrt(D)
  p = softmax(scores, axis=-1)                     -> output 2 [B, N, N]
  out_t = einsum('bnm,bdm->bdn', p, v)             -> output 1 [B, D, N]

Sharding: data-parallel over batch, 2 batches per core on 8 cores.

Per-core pipeline (per batch):
  - PE-transpose Q,K (fp32) into [d, n] layouts; V cast to bf16 and
    PE-transposed into [m, d] layout. Transposes are batched 4 per PSUM
    bank (one accumulation group) with a single wide copyback.
  - scores tiles [128q x 2048m] via fp32r matmuls (full PE rate) in PSUM.
  - exp on ScalarE with scale=1/16 and accum_out => row sums for free.
  - normalize: fp32 p (DVE tensor_scalar) -> DMA to p_attn; bf16 p on
    GPSIMD (otherwise idle) for the AV matmul.
  - PE-transpose bf16 p tiles into [m, n] strips (groups of 4 q-tiles),
    AV matmuls (bf16) accumulate over m -> out_t written in natural
    [d, n] orientation.
"""

import os
import sys

if "/opt/trn_rl_repo" not in sys.path:
    sys.path.insert(0, "/opt/trn_rl_repo")

import numpy as np

B, N, D = 16, 2048, 256
N_CORES = 8
B_LOC = B // N_CORES  # batches per core
P = 128
NT = N // P            # 16 q-tiles (or m-chunks) per batch
DT = D // P            # 2 d-chunks
GRP = 4                # q-tiles per AV group
SCALE = 1.0 / 16.0     # 1/sqrt(D)

_cached_nc = {}


def _build_nc(repeat=1, mode="full", avdt="bf16"):
    key = (repeat, mode, avdt)
    if key in _cached_nc:
        return _cached_nc[key]

    import concourse.bacc as bacc
    import concourse.tile as tile
    import concourse.mybir as mybir
    from concourse.masks import make_identity

    f32 = mybir.dt.float32
    f32r = mybir.dt.float32r
    bf16 = mybir.dt.bfloat16
    EXP = mybir.ActivationFunctionType.Exp

    nc = bacc.Bacc("TRN2", target_bir_lowering=False)

    q = nc.dram_tensor("q", [B_LOC, N, D], f32, kind="ExternalInput")
    k = nc.dram_tensor("k", [B_LOC, N, D], f32, kind="ExternalInput")
    v = nc.dram_tensor("v", [B_LOC, D, N], f32, kind="ExternalInput")
    out_t = nc.dram_tensor("out_t", [B_LOC, D, N], f32, kind="ExternalOutput")
    p_attn = nc.dram_tensor("p_attn", [B_LOC, N, N], f32, kind="ExternalOutput")

    with tile.TileContext(nc) as tc:
        with (
            tc.tile_pool(name="consts", bufs=1) as consts,
            tc.tile_pool(name="qk_cache", bufs=2) as qk_cache,
            tc.tile_pool(name="stage", bufs=2) as stage,
            tc.tile_pool(name="vstage", bufs=2) as vstage,
            tc.tile_pool(name="pp", bufs=(3 if avdt == "bf16" else 2)) as pp,
            tc.tile_pool(name="pnp", bufs=2) as pnp,
            tc.tile_pool(name="strips", bufs=(2 if avdt == "bf16" else 1)) as strips,
            tc.tile_pool(name="zp", bufs=4) as zp,
            tc.tile_pool(name="op", bufs=2) as op,
            tc.tile_pool(name="ps_pool", bufs=2, space="PSUM") as ps_pool,
            tc.tile_pool(name="pt_pool", bufs=2, space="PSUM") as pt_pool,
            tc.tile_pool(name="po_pool", bufs=2, space="PSUM") as po_pool,
        ):
            ident_f = consts.tile([P, P], f32)
            make_identity(nc, ident_f)
            ident_b = consts.tile([P, P], bf16)
            make_identity(nc, ident_b)
            ident_r = None
            if avdt == "f32r":
                ident_r = consts.tile([P, P], f32r)
                nc.vector.tensor_copy(ident_r, ident_f)

            import contextlib
            loop_ctx = (
                tc.For_i(0, repeat, 1) if repeat > 1 else contextlib.nullcontext()
            )

            def batched_transpose(dst_ap, srcs, ident, dtype):
                """Transpose 4 [P, P] SBUF blocks into one PSUM bank
                (single accumulation group), then one wide copyback."""
                nb = len(srcs)
                tp = pt_pool.tile([P, nb * P], dtype, tag="pt")
                for j, s in enumerate(srcs):
                    nc.tensor.matmul(
                        tp[:, j * P : (j + 1) * P],
                        lhsT=s,
                        rhs=ident,
                        is_transpose=True,
                        start=(j == 0),
                        stop=(j == nb - 1),
                    )
                nc.any.tensor_copy(dst_ap, tp)

            with loop_ctx:
              for b in range(B_LOC):
                # ---- input load + transposes ----
                qt = qk_cache.tile([P, DT, N], f32r, tag="qt")
                kt = qk_cache.tile([P, DT, N], f32r, tag="kt")
                vt = qk_cache.tile([P, NT, D], bf16 if avdt == "bf16" else f32r, tag="vt")

                for i0 in range(0, NT, 4):
                    qn4 = stage.tile([P, 4, D], f32, tag="qn")
                    nc.sync.dma_start(
                        qn4,
                        q[b, i0 * P : (i0 + 4) * P, :].rearrange(
                            "(t p) d -> p t d", p=P
                        ),
                    )
                    kn4 = stage.tile([P, 4, D], f32, tag="kn")
                    nc.sync.dma_start(
                        kn4,
                        k[b, i0 * P : (i0 + 4) * P, :].rearrange(
                            "(t p) d -> p t d", p=P
                        ),
                    )
                    for d in range(DT):
                        batched_transpose(
                            qt[:, d, i0 * P : (i0 + 4) * P],
                            [qn4[:, t, d * P : (d + 1) * P] for t in range(4)],
                            ident_f,
                            f32,
                        )
                        batched_transpose(
                            kt[:, d, i0 * P : (i0 + 4) * P],
                            [kn4[:, t, d * P : (d + 1) * P] for t in range(4)],
                            ident_f,
                            f32,
                        )

                for d in range(DT if mode in ("full", "noout") else 0):
                    vn = vstage.tile([P, N], f32, tag="vn")
                    nc.sync.dma_start(vn, v[b, d * P : (d + 1) * P, :])
                    vsrc = vstage.tile([P, N], bf16, tag="vb")
                    nc.vector.tensor_copy(vsrc, vn)
                    for mi0 in range(0, NT, 4):
                        batched_transpose(
                            vt[:, mi0 : mi0 + 4, d * P : (d + 1) * P],
                            [
                                vsrc[:, mi * P : (mi + 1) * P]
                                for mi in range(mi0, mi0 + 4)
                            ],
                            ident_b,
                            bf16,
                        )

                # ---- main loop over groups of q-tiles ----
                for g in range(NT // GRP):
                    strip = strips.tile([P, NT, GRP * P], bf16 if avdt == "bf16" else f32r, tag="strip")
                    for qi in range(GRP):
                        i = g * GRP + qi
                        p_sb = pp.tile([P, N], f32 if avdt == "bf16" else f32r, tag="p")
                        zpart = zp.tile([P, 2], f32, tag="zpart")
                        for mc in range(2):
                            ps = ps_pool.tile([P, N // 2], f32, tag="ps")
                            for half in range(2):
                                mo = mc * (N // 2) + half * 512
                                for d in range(DT):
                                    nc.tensor.matmul(
                                        ps[:, half * 512 : (half + 1) * 512],
                                        lhsT=qt[:, d, i * P : (i + 1) * P],
                                        rhs=kt[:, d, mo : mo + 512],
                                        start=(d == 0),
                                        stop=(d == DT - 1),
                                    )
                            nc.scalar.activation(
                                p_sb[:, mc * (N // 2) : (mc + 1) * (N // 2)],
                                ps,
                                EXP,
                                scale=SCALE,
                                accum_out=zpart[:, mc : mc + 1],
                            )
                        if mode == "qk":
                            continue
                        z_t = zp.tile([P, 1], f32, tag="z")
                        nc.scalar.add(z_t, zpart[:, 0:1], zpart[:, 1:2])
                        r_t = zp.tile([P, 1], f32, tag="r")
                        nc.vector.reciprocal(r_t, z_t)

                        if avdt == "bf16":
                            tsrc = pnp.tile([P, N], bf16, tag="pb")
                            nc.vector.tensor_scalar_mul(tsrc, p_sb, r_t)
                            tid2, tdt2 = ident_b, bf16
                        else:
                            tsrc = p_sb
                            tid2, tdt2 = ident_r, f32r
                        if mode != "noout" and mode != "qk":
                            nc.gpsimd.dma_start(
                                p_attn[b, i * P : (i + 1) * P, :], tsrc
                            )
                        if mode == "noav":
                            continue
                        for mi0 in range(0, NT, 4):
                            batched_transpose(
                                strip[:, mi0 : mi0 + 4, qi * P : (qi + 1) * P],
                                [
                                    tsrc[:, mi * P : (mi + 1) * P]
                                    for mi in range(mi0, mi0 + 4)
                                ],
                                tid2,
                                tdt2,
                            )

                    for d in range(DT if mode in ("full", "noout") else 0):
                        po = po_pool.tile([P, GRP * P], f32, tag="po")
                        for mi in range(NT):
                            nc.tensor.matmul(
                                po,
                                lhsT=vt[:, mi, d * P : (d + 1) * P],
                                rhs=strip[:, mi, :],
                                start=(mi == 0),
                                stop=(mi == NT - 1),
                            )
                        o_sb = op.tile([P, GRP * P], f32, tag="o")
                        nc.any.tensor_copy(o_sb, po)
                        nc.sync.dma_start(
                            out_t[
                                b,
                                d * P : (d + 1) * P,
                                g * GRP * P : (g + 1) * GRP * P,
                            ],
                            o_sb,
                        )

    nc.compile()
    _cached_nc[key] = nc
    return nc


def kernel(query, key, value, mask=0):
    from concourse.bass_utils import run_bass_kernel_spmd

    nc = _build_nc()

    query = np.ascontiguousarray(np.asarray(query, dtype=np.float32))
    key = np.ascontiguousarray(np.asarray(key, dtype=np.float32))
    value = np.ascontiguousarray(np.asarray(value, dtype=np.float32))

    in_maps = []
    for c in range(N_CORES):
        sl = slice(c * B_LOC, (c + 1) * B_LOC)
        in_maps.append(
            {
                "q": query[sl],
                "k": key[sl],
                "v": value[sl],
            }
        )

    trace = os.environ.get("ATT_TRACE", "0") == "1"
    res = run_bass_kernel_spmd(
        nc, in_maps, core_ids=list(range(N_CORES)), trace=trace
    )
    if trace and res.exec_time_ns is not None:
        print(f"HW exec time: {res.exec_time_ns} ns")
        kernel.last_exec_time_ns = res.exec_time_ns

    out_t = np.concatenate([res.results[c]["out_t"] for c in range(N_CORES)], axis=0)
    p_attn = np.concatenate(
        [res.results[c]["p_attn"] for c in range(N_CORES)], axis=0
    )
    return out_t, p_attn
